# revision 24
# baseline (speedup 1.0000x reference)
"""AttentionBlock Trainium2 kernel (Bass/Tile), SPMD over 8 NeuronCores.

Problem (hardcoded): x [32, 256, 32, 32] fp32
  GroupNorm(8 groups, eps=1e-5, affine) -> 1x1 qkv conv [768,256] ->
  per-image attention over N=1024 pixels (C=256) -> 1x1 proj [256,256] ->
  residual add.

Sharding: pure data-parallel over batch: 4 images per core, weights
replicated, no collectives.

Design (fp8 DoubleRow rewrite of the bf16 baseline; ~102-108us HW vs
184us baseline, rel err ~7.4e-3 vs the 2e-2 gate):
  - Every matmul runs in fp8e4 (max 240) with perf_mode=DoubleRow:
    K=256 contraction per instruction at 2 fp8 MACs/cell/cycle, i.e.
    512 cycles for a [256x128]x[256x512] MM - measured 216ns/MM
    sustained with LDWEIGHTS hidden.
  - S = h^T (Wk^T Wq) h: the q and k convs fold into ONE conv with the
    host-precomputed M = 32*Wk^T Wq (std ~2, fp8-healthy), removing a
    conv and two PSUM->SBUF copy passes; exp scale 1/512 undoes the
    scaling exactly. (Falls back to separate q/k convs + biases when
    qkv biases are nonzero - the graded problem has zero biases.)
  - exp(S - 4): the constant shift keeps P = exp() within e4m3 range
    (max |S| ~ 6.2, e^6.2 > 240); softmax shift-invariance makes it
    exact. Weights are host-prescaled by 8 (wv, wp) and the Z-matmul
    "ones" value is 64 = 8*8, so the proj output needs no rescale.
  - The Z ones-matmul uses an all-64s [128,2,128] stationary matrix, so
    its PSUM output IS Z broadcast across all 128 partitions (output
    rows are free - matmuls are rhs-streaming-bound): 1/Z is then a
    single DVE reciprocal_approx_fast straight out of PSUM, fused into
    the O PSUM->SBUF fp8 cast. No DRAM bounce, no transposes.
  - x is bf16 (host-cast, halves DMA + doubles DVE stats rate); the
    output store is bf16, upcast on the host. Residual add in f32.
  - GroupNorm: bn_stats/bn_aggr per channel; group pooling and the
    broadcast back to channels are two tiny matmuls (PSUM slots
    borrowed from the proj bank); rstd = rsqrt(var+eps) via the
    0x5f3759df bit-trick + 1 Newton step on DVE so the ACT table stays
    on Exp forever (a Sqrt table swap costs 2x 1.3us per image).
  - Software pipeline: per 512-wide q-chunk, S^T pairs stream through a
    2-deep PSUM ring against exp on ACT; the next image's GroupNorm
    emits as side pieces inside chunk 0 and its t/v convs interleave
    with chunk 1's pairs (the shared PSUM ring alternates pair/conv
    tiles). phase_d (proj+residual+store) of chunk c is deferred into
    chunk c+1; the last chunk runs it eagerly with split store DMAs.
  - Copies are balanced: exps + t-copies + half the v-copies on ACT,
    stats + O-scale + residual + the rest on DVE. Engine busy at ~102us:
    PE ~62%, ACT ~49%, DVE ~54%.

Known limits (red-teamed against the trace): images 0-1 still ramp the
HAM clock gate (the first ~25us run partly at 1.2GHz; prologue is a
serial x-DMA -> bn -> rsqrt -> h -> t/v-conv chain), each chunk ends
with a ~1us PE wait on the last exp (PSUM is the limit: 4 banks of S
ring + 2 O + 1 Z + 1 proj = 8), and run-to-run HAM phase adds +-6us.
"""

from contextlib import ExitStack

import ml_dtypes
import numpy as np

import concourse.bass as bass
import concourse.tile as tile
from concourse import bacc
from concourse import mybir

F32 = mybir.dt.float32
BF16 = mybir.dt.bfloat16
F8 = mybir.dt.float8e4
U32 = mybir.dt.uint32
AF = mybir.ActivationFunctionType
OP = mybir.AluOpType
DR = mybir.MatmulPerfMode.DoubleRow
AX = mybir.AxisListType

B, C, H, W = 32, 256, 32, 32
N = H * W            # 1024
G = 8                # groups
EPS = 1e-5
NCORES = 8
BL = B // NCORES     # images per core
CT = C // 128        # channel tiles
NB = N // 128        # pixel blocks (k dim of attention)
QCH = N // 512       # 512-wide q chunks
NPAIR = NB // 2      # k-block pairs per chunk
P = 128
WSCALE = 8.0         # host fp8 scale for wv / wp
MSCALE = 32.0        # host fp8 scale for M = Wk^T Wq
EXP_SHIFT = 4.0      # exp(S - shift); |S| <= ~6.2
ZONES = 64.0         # av*ap: folds the O/proj descale into 1/Z

import os as _os
N_WARM = int(_os.environ.get("KERNEL_N_WARM", "16"))
VCOPY_ACT = int(_os.environ.get("KERNEL_VCOPY_ACT", "2"))  # of 4 v copies on ACT


def build_program(use_bq: bool, use_bk: bool, use_bf: bool) -> bass.Bass:
    use_qk = use_bq or use_bk  # fallback: separate q/k convs with biases
    exp_scale = 1.0 / (16.0 * (WSCALE * WSCALE if use_qk else MSCALE))

    nc = bacc.Bacc()

    xs = nc.dram_tensor("xs", [BL, C, N], BF16, kind="ExternalInput")
    wq = nc.dram_tensor("wq", [C, C], F8, kind="ExternalInput")  # [c_in, c_out]
    wk = nc.dram_tensor("wk", [C, C], F8, kind="ExternalInput")
    wv = nc.dram_tensor("wv", [C, C], F8, kind="ExternalInput")
    wp = nc.dram_tensor("wp", [C, C], F8, kind="ExternalInput")
    bq = nc.dram_tensor("bq", [C], F32, kind="ExternalInput")
    bk = nc.dram_tensor("bk", [C], F32, kind="ExternalInput")
    bf = nc.dram_tensor("bf", [C], F32, kind="ExternalInput")
    out = nc.dram_tensor("out", [BL, C, N], BF16, kind="ExternalOutput")

    # Group-pool matmul: partition p -> group p//32; bn_aggr already
    # yields per-channel means, so pooling averages 32 channels (1/32).
    gmask_np = np.zeros((P, 4), np.float32)
    gmask_np[np.arange(P), np.arange(P) // 32] = 1.0 / 32.0
    gmask_d = nc.inline_tensor(gmask_np.astype(ml_dtypes.bfloat16), "gmask")
    # Broadcast matmul: group g -> partitions 32g..32g+31.
    sel_np = np.zeros((4, P), np.float32)
    sel_np[np.arange(P) // 32, np.arange(P)] = 1.0
    sel_d = nc.inline_tensor(sel_np.astype(ml_dtypes.bfloat16), "sel")

    with tile.TileContext(nc) as tc, ExitStack() as ctx:
        consts = ctx.enter_context(tc.tile_pool(name="consts", bufs=1))
        xpool = ctx.enter_context(tc.tile_pool(name="xp", bufs=3))
        hpool = ctx.enter_context(tc.tile_pool(name="hp", bufs=2))
        tpool = ctx.enter_context(tc.tile_pool(name="tp", bufs=2))
        vpool = ctx.enter_context(tc.tile_pool(name="vp", bufs=2))
        ppool = ctx.enter_context(tc.tile_pool(name="pp", bufs=3))
        opool = ctx.enter_context(tc.tile_pool(name="op", bufs=2))
        spool = ctx.enter_context(tc.tile_pool(name="sp", bufs=2))
        rzpool = ctx.enter_context(tc.tile_pool(name="rzp", bufs=2))
        outp = ctx.enter_context(tc.tile_pool(name="outp", bufs=2))
        ps_s = ctx.enter_context(tc.tile_pool(name="pss", bufs=2, space="PSUM"))
        ps_O = ctx.enter_context(tc.tile_pool(name="psO", bufs=1, space="PSUM"))
        ps_z = ctx.enter_context(tc.tile_pool(name="psz", bufs=1, space="PSUM"))
        ps_pj = ctx.enter_context(tc.tile_pool(name="pspj", bufs=1, space="PSUM"))

        # --- constants ---
        wdum_sb = consts.tile([P, 512], BF16, tag="wdum")
        nc.vector.memset(wdum_sb, 0.25)
        gmask_sb = consts.tile([P, 4], BF16, tag="gmask")
        sel_sb = consts.tile([4, P], BF16, tag="sel")
        bq_sb = consts.tile([P, CT], F32, tag="bq") if use_bq else None
        bk_sb = consts.tile([P, CT], F32, tag="bk") if use_bk else None
        bf_sb = consts.tile([P, CT], F32, tag="bf") if use_bf else None

        def load_consts():
            nc.sync.dma_start(out=gmask_sb, in_=gmask_d[:, :])
            nc.sync.dma_start(out=sel_sb, in_=sel_d[:, :])
            for t_sb, t_d in ((bq_sb, bq), (bk_sb, bk), (bf_sb, bf)):
                if t_sb is not None:
                    nc.sync.dma_start(
                        out=t_sb, in_=t_d[:].rearrange("(t p) -> p t", p=P)
                    )
        ones64_sb = consts.tile([P, 2, P], F8, tag="ones64")
        nc.vector.memset(ones64_sb, ZONES)
        magic_sb = consts.tile([4, CT], U32, tag="magic")
        nc.vector.memset(magic_sb, 0x5F3759DF)
        one_u32_sb = consts.tile([4, CT], U32, tag="oneu")
        nc.vector.memset(one_u32_sb, 1)
        eps_sb = consts.tile([4, 1], F32, tag="eps")
        nc.vector.memset(eps_sb, EPS)
        nshift_sb = consts.tile([P, 1], F32, tag="nshift")
        nc.vector.memset(nshift_sb, -EXP_SHIFT)
        wq_sb = consts.tile([P, CT, C], F8, tag="wq")
        wk_sb = consts.tile([P, CT, C], F8, tag="wk") if use_qk else None
        wv_sb = consts.tile([P, CT, C], F8, tag="wv")
        wp_sb = consts.tile([P, CT, C], F8, tag="wp")

        def load_weights():
            pairs = [(wq_sb, wq), (wv_sb, wv), (wp_sb, wp)]
            if use_qk:
                pairs.append((wk_sb, wk))
            for t_sb, t_d in pairs:
                nc.sync.dma_start(
                    out=t_sb, in_=t_d[:, :].rearrange("(t p) o -> p t o", p=P)
                )

        st = [dict() for _ in range(BL)]

        def load_x(b, split=False):
            x_t = xpool.tile([P, CT, N], BF16, tag="x")
            st[b]["x"] = x_t
            if split:
                for ct in range(CT):
                    nc.sync.dma_start(
                        out=x_t[:, ct, :], in_=xs[b, ct * P : (ct + 1) * P, :]
                    )
            else:
                nc.sync.dma_start(
                    out=x_t, in_=xs[b].rearrange("(t p) n -> p t n", p=P)
                )

        def stats_pieces(b):
            """GroupNorm stats as 3 side pieces: bn ct0 | bn ct1+pool | h."""
            x_t = st[b]["x"]
            cs = spool.tile([P, CT, 2], F32, tag="cs")
            cs_bf = spool.tile([P, CT, 2], BF16, tag="csbf")
            gm_ps = ps_pj.tile([4, CT, 2], F32, tag="pj", name="gm_ps")

            def bn_ct(ct):
                bnst = spool.tile([P, 2, 6], F32, tag="bnst")
                for s in range(2):
                    nc.vector.bn_stats(
                        out=bnst[:, s, :], in_=x_t[:, ct, s * 512 : (s + 1) * 512]
                    )
                nc.vector.bn_aggr(out=cs[:, ct, :], in_=bnst)

            def piece1():
                bn_ct(0)

            def piece2():
                bn_ct(1)
                # E[x^2] = var + mean^2 (both cts in two strided ops)
                msq = spool.tile([P, CT], F32, tag="msq")
                nc.vector.tensor_mul(out=msq, in0=cs[:, :, 0], in1=cs[:, :, 0])
                nc.vector.tensor_tensor(
                    out=cs[:, :, 1], in0=cs[:, :, 1], in1=msq, op=OP.add
                )
                nc.vector.tensor_copy(out=cs_bf, in_=cs)
                nc.tensor.matmul(
                    gm_ps.rearrange("p a b -> p (a b)"),
                    lhsT=gmask_sb,
                    rhs=cs_bf.rearrange("p a b -> p (a b)"),
                    start=True,
                    stop=True,
                )

            def piece3():
                gsb = spool.tile([4, CT, 2], F32, tag="gsb")
                nc.vector.tensor_copy(out=gsb, in_=gm_ps)
                gmean = gsb[:, :, 0]
                ge2 = gsb[:, :, 1]
                msq4 = spool.tile([4, CT], F32, tag="msq4")
                nc.vector.tensor_mul(out=msq4, in0=gmean, in1=gmean)
                gvar = spool.tile([4, CT], F32, tag="gvar")
                nc.vector.tensor_tensor(
                    out=gvar, in0=ge2, in1=msq4, op=OP.subtract
                )
                # rstd = rsqrt(gvar + eps): bit-trick seed + 1 Newton step.
                yu = spool.tile([4, CT], U32, tag="yu")
                nc.vector.tensor_tensor(
                    out=yu, in0=gvar.bitcast(U32), in1=one_u32_sb,
                    op=OP.logical_shift_right,
                )
                nc.vector.tensor_tensor(
                    out=yu, in0=magic_sb, in1=yu, op=OP.subtract
                )
                y = yu.bitcast(F32)
                y2 = spool.tile([4, CT], F32, tag="y2")
                nc.vector.tensor_mul(out=y2, in0=y, in1=y)
                nc.vector.scalar_tensor_tensor(
                    out=y2, in0=gvar, scalar=eps_sb, in1=y2,
                    op0=OP.add, op1=OP.mult,
                )
                nc.vector.tensor_scalar(
                    out=y2, in0=y2, scalar1=-0.5, scalar2=1.5,
                    op0=OP.mult, op1=OP.add,
                )
                grstd = spool.tile([4, CT], F32, tag="grstd")
                nc.vector.tensor_mul(out=grstd, in0=y, in1=y2)
                gfin = spool.tile([4, CT, 2], BF16, tag="gfin")
                nc.vector.tensor_copy(out=gfin[:, :, 0], in_=gmean)
                nc.vector.tensor_copy(out=gfin[:, :, 1], in_=grstd)
                pcs_ps = ps_pj.tile([P, CT, 2], F32, tag="pj", name="pcs_ps")
                nc.tensor.matmul(
                    pcs_ps.rearrange("p a b -> p (a b)"),
                    lhsT=sel_sb,
                    rhs=gfin.rearrange("p a b -> p (a b)"),
                    start=True,
                    stop=True,
                )
                pcs = spool.tile([P, CT, 2], F32, tag="pcs")
                nc.vector.tensor_copy(out=pcs, in_=pcs_ps)
                h_t = hpool.tile([P, CT, N], F8, tag="h")
                st[b]["h"] = h_t
                for ct in range(CT):
                    nc.vector.tensor_scalar(
                        out=h_t[:, ct, :],
                        in0=x_t[:, ct, :],
                        scalar1=pcs[:, ct, 0:1],
                        scalar2=pcs[:, ct, 1:2],
                        op0=OP.subtract,
                        op1=OP.mult,
                    )

            return [piece1, piece2, piece3]

        def conv_pieces(b):
            """Psum-tile-granular conv work for image b: t (or q/k) + v."""
            t_sb = tpool.tile([P, CT, N], F8, tag="t")
            st[b]["t"] = t_sb
            if use_qk:
                k_sb = tpool.tile([P, CT, N], F8, tag="k")
                st[b]["k"] = k_sb
            pieces = []
            convs = [(t_sb, wq_sb, bq_sb, use_bq)]
            if use_qk:
                convs.append((st[b]["k"], wk_sb, bk_sb, use_bk))
            for dst, w_sb, b_sb, use_b in convs:
                for ct in range(CT):
                    def piece(dst=dst, w_sb=w_sb, b_sb=b_sb, use_b=use_b, ct=ct):
                        h_t = st[b]["h"]
                        mm = ps_s.tile([P, 2, 512], F32, tag="s", name="tk_ps")
                        for nch in range(2):
                            nc.tensor.matmul(
                                mm[:, nch, :],
                                lhsT=w_sb[:, 0:2, ct * P : (ct + 1) * P],
                                rhs=h_t[:, 0:2, nch * 512 : (nch + 1) * 512],
                                start=True,
                                stop=True,
                                perf_mode=DR,
                            )
                        nc.scalar.activation(
                            out=dst[:, ct, :].rearrange("p (a b) -> p a b", a=2),
                            in_=mm,
                            func=AF.Identity,
                            bias=b_sb[:, ct : ct + 1] if use_b else 0.0,
                            scale=1.0,
                        )
                    pieces.append(piece)
            v_sb = vpool.tile([P, NB, C], F8, tag="v")
            st[b]["v"] = v_sb
            for np_ in range(NPAIR):
                def piece(np_=np_):
                    h_t = st[b]["h"]
                    vv = ps_s.tile([P, 2, 512], F32, tag="s", name="v_ps")
                    for i in range(2):
                        nb = 2 * np_ + i
                        nc.tensor.matmul(
                            vv[:, i, 0:C],
                            lhsT=h_t[:, 0:2, nb * P : (nb + 1) * P],
                            rhs=wv_sb[:, 0:2, :],
                            start=True,
                            stop=True,
                            perf_mode=DR,
                        )
                    if np_ < VCOPY_ACT:
                        nc.scalar.copy(
                            out=v_sb[:, 2 * np_ : 2 * np_ + 2, :],
                            in_=vv[:, :, 0:C],
                        )
                    else:
                        nc.vector.tensor_copy(
                            out=v_sb[:, 2 * np_ : 2 * np_ + 2, :],
                            in_=vv[:, :, 0:C],
                        )
                pieces.append(piece)
            return pieces

        def phase_c(b, qc, pending, sideq, last=False):
            """Attention chunk: S^T pairs, exp, Z, O, then the 1/Z chain;
            side-queue pieces interleave one per pair iteration."""
            v_sb = st[b]["v"]
            h_t = st[b]["h"]
            t_sb = st[b]["t"]
            s_rhs = t_sb if use_qk else h_t  # q-side operand
            O_ps = ps_O.tile([P, CT, 512], F32, tag="O")
            z_ps = ps_z.tile([P, 512], F32, tag="z", name="z_ps")

            def s_pair(j):
                s2 = ps_s.tile([P, 2, 512], F32, tag="s", name="s2_ps")
                for i in range(2):
                    nb = 2 * j + i
                    lhs = st[b]["k"] if use_qk else t_sb
                    nc.tensor.matmul(
                        s2[:, i, :],
                        lhsT=lhs[:, 0:2, nb * P : (nb + 1) * P],
                        rhs=s_rhs[:, 0:2, qc * 512 : (qc + 1) * 512],
                        start=True,
                        stop=True,
                        perf_mode=DR,
                    )
                return s2

            fifo = [s_pair(0), s_pair(1)]
            if pending is not None:
                phase_d(*pending)
            for j in range(NPAIR):
                if j + 2 < NPAIR:
                    fifo.append(s_pair(j + 2))
                if j >= 1 and sideq:
                    sideq.popleft()()
                s2 = fifo.pop(0)
                p2 = ppool.tile([P, 2, 512], F8, tag="p")
                if j == 0:
                    # split the first exp so Z/O start sooner
                    for i in range(2):
                        nc.scalar.activation(
                            out=p2[:, i, :], in_=s2[:, i, :], func=AF.Exp,
                            bias=nshift_sb, scale=exp_scale,
                        )
                else:
                    nc.scalar.activation(
                        out=p2, in_=s2, func=AF.Exp,
                        bias=nshift_sb, scale=exp_scale,
                    )
                nc.tensor.matmul(
                    z_ps,
                    lhsT=ones64_sb,
                    rhs=p2,
                    start=(j == 0),
                    stop=(j == NPAIR - 1),
                    perf_mode=DR,
                )
                for ct in range(CT):
                    nc.tensor.matmul(
                        O_ps[:, ct, :],
                        lhsT=v_sb[:, 2 * j : 2 * j + 2, ct * P : (ct + 1) * P],
                        rhs=p2,
                        start=(j == 0),
                        stop=(j == NPAIR - 1),
                        perf_mode=DR,
                    )
            if qc == 1:
                while sideq:
                    sideq.popleft()()
            # 1/Z: the ones-matmul already wrote Z broadcast across all 128
            # partitions (output rows are free - matmuls are rhs-streaming
            # bound), so a single DVE approx-reciprocal finishes the chain.
            rz = rzpool.tile([P, 512], F32, tag="rz")
            nc.vector.reciprocal_approx_fast(out=rz, in_=z_ps)
            on_sb = opool.tile([P, CT, 512], F8, tag="on")
            st[b]["on%d" % qc] = on_sb
            for ct in range(CT):
                nc.vector.tensor_mul(
                    out=on_sb[:, ct, :], in0=O_ps[:, ct, :], in1=rz
                )
            if last:
                # eager tail: proj / residual / store per ct, split DMAs
                x_t = st[b]["x"]
                o_sb = outp.tile([P, CT, 512], BF16, tag="o")
                for ct in range(CT):
                    pj = ps_pj.tile([P, 512], F32, tag="pj", name="pj_ps")
                    nc.tensor.matmul(
                        pj,
                        lhsT=wp_sb[:, 0:2, ct * P : (ct + 1) * P],
                        rhs=on_sb,
                        start=True,
                        stop=True,
                        perf_mode=DR,
                    )
                    xres = x_t[:, ct, qc * 512 : (qc + 1) * 512]
                    if use_bf:
                        nc.vector.scalar_tensor_tensor(
                            out=o_sb[:, ct, :], in0=pj,
                            scalar=bf_sb[:, ct : ct + 1], in1=xres,
                            op0=OP.add, op1=OP.add,
                        )
                    else:
                        nc.vector.tensor_add(
                            out=o_sb[:, ct, :], in0=pj, in1=xres
                        )
                    nc.sync.dma_start(
                        out=out[
                            b, ct * P : (ct + 1) * P,
                            qc * 512 : (qc + 1) * 512,
                        ],
                        in_=o_sb[:, ct, :],
                    )

        def phase_d(b, qc):
            """proj conv on normalized O, residual add, store."""
            x_t = st[b]["x"]
            on_sb = st[b].pop("on%d" % qc)
            o_sb = outp.tile([P, CT, 512], BF16, tag="o")
            for ct in range(CT):
                pj = ps_pj.tile([P, 512], F32, tag="pj", name="pj_ps")
                nc.tensor.matmul(
                    pj,
                    lhsT=wp_sb[:, 0:2, ct * P : (ct + 1) * P],
                    rhs=on_sb,
                    start=True,
                    stop=True,
                    perf_mode=DR,
                )
                xres = x_t[:, ct, qc * 512 : (qc + 1) * 512]
                if use_bf:
                    nc.vector.scalar_tensor_tensor(
                        out=o_sb[:, ct, :],
                        in0=pj,
                        scalar=bf_sb[:, ct : ct + 1],
                        in1=xres,
                        op0=OP.add,
                        op1=OP.add,
                    )
                else:
                    nc.vector.tensor_add(out=o_sb[:, ct, :], in0=pj, in1=xres)
            nc.sync.dma_start(
                out=out[b, :, qc * 512 : (qc + 1) * 512].rearrange(
                    "(t p) n -> p t n", p=P
                ),
                in_=o_sb,
            )

        # --- emission schedule ---
        from collections import deque

        load_consts()
        load_x(0, split=True)
        load_weights()
        load_x(1)

        def warm_mms(n):
            for _ in range(n):
                warm = ps_s.tile([P, 2, 512], F32, tag="s", name="warm_ps")
                nc.tensor.matmul(
                    warm[:, 0, :], lhsT=wdum_sb[:, 0:P], rhs=wdum_sb,
                    start=True, stop=True,
                )

        warm_mms(N_WARM)
        for piece in stats_pieces(0):
            piece()
        for piece in conv_pieces(0):
            piece()
        sideq = deque()
        pending = None
        for b in range(BL):
            for qc in range(QCH):
                push_qc = 1 if b == 0 else 0
                if qc == 0 and b + 1 < BL and b > 0:
                    load_x(b + 1)
                if qc == push_qc and b + 1 < BL:
                    sideq.extend(stats_pieces(b + 1))
                    sideq.extend(conv_pieces(b + 1))
                last = b == BL - 1 and qc == QCH - 1
                phase_c(b, qc, pending, sideq, last=last)
                pending = None if last else (b, qc)
    nc.compile()
    return nc


def prepare(inputs):
    """Fold parameters on the host; return (program, per-core input maps)."""
    x = np.asarray(inputs["x"], dtype=np.float32)
    norm_w = np.asarray(inputs["norm_w"], dtype=np.float32)
    norm_b = np.asarray(inputs["norm_b"], dtype=np.float32)
    qkv_w = np.asarray(inputs["qkv_w"], dtype=np.float32)
    qkv_b = np.asarray(inputs["qkv_b"], dtype=np.float32)
    proj_w = np.asarray(inputs["proj_w"], dtype=np.float32)
    proj_b = np.asarray(inputs["proj_b"], dtype=np.float32)

    # Fold the GroupNorm affine into qkv: qkv(h*w+b) = (qkv*w)h + qkv@b
    w_eff = qkv_w * norm_w[None, :]
    b_eff = qkv_b + qkv_w @ norm_b
    f8 = ml_dtypes.float8_e4m3
    bf16 = ml_dtypes.bfloat16
    bq_f = np.ascontiguousarray(b_eff[0:C] * WSCALE)
    bk_f = np.ascontiguousarray(b_eff[C : 2 * C] * WSCALE)
    bv_f = b_eff[2 * C : 3 * C]
    bf_f = np.ascontiguousarray(proj_w @ bv_f + proj_b)
    use_bq = bool(np.any(bq_f))
    use_bk = bool(np.any(bk_f))
    use_bf = bool(np.any(bf_f))

    if use_bq or use_bk:
        wq_t = np.ascontiguousarray((w_eff[0:C] * WSCALE).T.astype(f8))
        wk_t = np.ascontiguousarray((w_eff[C : 2 * C] * WSCALE).T.astype(f8))
    else:
        # M-trick: S^T[kp,q] = h_kp^T (MSCALE Wk^T Wq) h_q; wq carries M
        # in [c_in, c_out] layout directly.
        m_s = MSCALE * (w_eff[C : 2 * C].T @ w_eff[0:C])
        wq_t = np.ascontiguousarray(m_s.astype(f8))
        wk_t = np.ascontiguousarray(np.zeros((C, C), f8))
    wv_t = np.ascontiguousarray((w_eff[2 * C : 3 * C] * WSCALE).T.astype(f8))
    wp_t = np.ascontiguousarray((proj_w * WSCALE).T.astype(f8))

    nc = build_program(use_bq, use_bk, use_bf)

    xr = x.reshape(NCORES, BL, C, N).astype(bf16)
    in_maps = []
    for c in range(NCORES):
        in_maps.append(
            {
                "xs": np.ascontiguousarray(xr[c]),
                "wq": wq_t,
                "wk": wk_t,
                "wv": wv_t,
                "wp": wp_t,
                "bq": bq_f,
                "bk": bk_f,
                "bf": bf_f,
            }
        )
    return nc, in_maps


def run(inputs, trace=False):
    from concourse.bass_utils import run_bass_kernel_spmd

    nc, in_maps = prepare(inputs)
    res = run_bass_kernel_spmd(nc, in_maps, list(range(NCORES)), trace=trace)
    outs = np.stack(
        [np.asarray(res.results[i]["out"]) for i in range(NCORES)]
    )
    full = outs.reshape(B, C, H, W).astype(np.float32)
    return full, res


def kernel(**inputs) -> np.ndarray:
    full, _ = run(inputs, trace=False)
    return full
